# revision 39
# baseline (speedup 1.0000x reference)
"""Trainium2 Bass kernel for nn_Attention_46901042872408.

Dense MHA transformer block with RoPE + prefix-tuning branch:
  q/k/v = x @ wq/wk/wv; rope(q), rope(k); causal attention;
  prefix branch: non-causal attention of q against (prefix @ wk/wv),
  gated by tanh(prefix_gate) per head; out = (attn + gate*prefix_attn) @ wo.

Sharding: 8 cores = data-parallel over batch (2) x tensor-parallel over
heads (4 groups of 8 heads). Each core computes a partial [2048, 4096]
output (its heads' contribution through its wo row-slice); host sums the
4 partials per batch.

All matmul operands bf16 (PSUM accumulates f32); tolerance is 2e-2 so
bf16's ~1e-3 matmul error is fine and it halves DMA + SBUF.

Key structure (instruction/DMA count minimized; ~0.8ms of the chained
measurement is fixed per-exec dispatch overhead that hides most of
phase 1, so phases 2+3 are the exposed cost):
  Phase 1: x-stationary projections over 2 chunks of 1024 tokens,
    streaming host-pre-tiled 512-col weight blocks (contiguous 32KB
    per-partition DMA runs). wq/wk columns host-permuted to
    even/odd-split order within each head so RoPE is 4 contiguous
    free-dim DVE ops against resident factor tables; q/k PE-transposed
    to [hd, token], accumulated to [128, 1024] tiles, spilled in 2KB
    rows; v spilled straight in [tok%128, tok//128, col] layout.
    Prefix k/v projections ride the same weight stream.
  Phase 2: v loaded once for all heads (one contiguous DMA); per head:
    kT + q loaded once; per (head, 512-token q-block): scores^T [k,q]
    tiles, exp on ACT (bf16 out), causal mask multiply on DVE only for
    diagonal tiles, PV + ones-denominator matmuls accumulated in PSUM,
    prefix branch with 30 k-rows, combine via reciprocal + gpsimd
    partition-broadcast; attention output stays SBUF-resident.
  Phase 3: out = attnT.T @ wo with wo SBUF-resident (loaded during
    phase 2), accumulated to [128, 4096] f32 row-blocks, 16 writes.

Rejected via measurement: gpsimd C-reduce for the softmax denominator
(65us/tile on HW, 13x regression), DMA-transpose loads + Pool-queue DMA
issue + Pool mask-multiplies (+11%), fp8 (error budget), PSUM-direct
DRAM writes (unsupported).
"""

import sys

sys.path.insert(0, "/opt/trn_rl_repo")

import numpy as np

B, S, D = 2, 2048, 4096
H, HD = 32, 128
PFX = 30
NCORES = 8
CPB = 4  # cores per batch (head-parallel groups)
HPC = 8  # heads per core
COLS = HPC * HD  # 1024 qkv columns per core
WB_COLS = 512  # weight column-block
NWB = 3 * COLS // WB_COLS  # 6 weight blocks: 0-1 q, 2-3 k, 4-5 v
NKT = D // 128  # 32 contraction tiles
CHUNK = 1024
CHUNKS = [(0, CHUNK), (CHUNK, CHUNK)]
SCALE = 1.0 / float(np.sqrt(HD))

_CACHE = {}


def _build(mm_fp32r=True):
    import os
    from contextlib import ExitStack

    phases = os.environ.get("KPHASES", "123")

    def knob(name, default):
        return int(os.environ.get(name, default))

    import concourse.tile as tile
    from concourse import bacc, mybir

    f32 = mybir.dt.float32
    mdt = mybir.dt.bfloat16
    AF = mybir.ActivationFunctionType
    OP = mybir.AluOpType

    nc = bacc.Bacc("TRN2", target_bir_lowering=False, debug=False, num_devices=NCORES)

    # host-pre-tiled inputs (contiguous per-partition DMA runs)
    xt = nc.dram_tensor("xt", [128, 2, NKT, CHUNK], mdt, kind="ExternalInput")
    wt = nc.dram_tensor("wt", [128, NWB, NKT, WB_COLS], mdt, kind="ExternalInput")
    wot = nc.dram_tensor("wot", [128, COLS // 128, D], mdt, kind="ExternalInput")
    pft = nc.dram_tensor("pft", [128, NKT, PFX], mdt, kind="ExternalInput")
    # rope factor tables in [tok%128, tok//128, col-pattern] layout
    s1_d = nc.dram_tensor("s1", [128, S // 128, WB_COLS], mdt, kind="ExternalInput")
    s2_d = nc.dram_tensor("s2", [128, S // 128, WB_COLS], mdt, kind="ExternalInput")
    masks = nc.dram_tensor("masks", [128, 4, 512], mdt, kind="ExternalInput")
    ones_d = nc.dram_tensor("ones", [128, 1], mdt, kind="ExternalInput")
    eye_d = nc.dram_tensor("eye", [128, 128], mdt, kind="ExternalInput")
    g_d = nc.dram_tensor("g", [1, HPC], f32, kind="ExternalInput")
    out_d = nc.dram_tensor("out", [S, D], f32, kind="ExternalOutput")

    with tile.TileContext(nc) as tc:
        with ExitStack() as top:
            dram = top.enter_context(tc.tile_pool(name="dram", bufs=1, space="DRAM"))
            qkT_sp = dram.tile([2 * COLS, S], mdt)  # [col, tok]; q rows 0..1023
            v_sp = dram.tile([128, S // 128, COLS], mdt)  # [tok%128, tok//128, col]

            pres = top.enter_context(tc.tile_pool(name="res", bufs=1))
            pf_sb = pres.tile([128, NKT, PFX], mdt)
            nc.sync.dma_start(pf_sb[:], pft[:])
            eye_sb = pres.tile([128, 128], mdt)
            nc.sync.dma_start(eye_sb[:], eye_d[:])
            ones_sb = pres.tile([128, 1], mdt)
            nc.sync.dma_start(ones_sb[:], ones_d[:])
            g_sb = pres.tile([1, HPC], f32)
            nc.sync.dma_start(g_sb[:], g_d[:])
            pkT_sb = pres.tile([128, HPC, PFX], mdt)
            pv_sb = pres.tile([PFX, 2, WB_COLS], mdt)  # straight prefix-v, 4 heads/blk

            # ---------------- Phase 1: projections ----------------
            with ExitStack() as ph1:
              if "1" in phases:
                pcs = ph1.enter_context(tc.tile_pool(name="pcs", bufs=1))
                s1_sb = pcs.tile([128, S // 128, WB_COLS], mdt)
                nc.sync.dma_start(s1_sb[:], s1_d[:])
                s2_sb = pcs.tile([128, S // 128, WB_COLS], mdt)
                nc.sync.dma_start(s2_sb[:], s2_d[:])
                px = ph1.enter_context(tc.tile_pool(name="px", bufs=1))
                pw = ph1.enter_context(tc.tile_pool(name="pw", bufs=knob("B_pw", 2)))
                ptmp = ph1.enter_context(tc.tile_pool(name="ptmp", bufs=knob("B_ptmp", 2)))
                po = ph1.enter_context(tc.tile_pool(name="po", bufs=knob("B_po", 3)))
                poT = ph1.enter_context(tc.tile_pool(name="poT", bufs=knob("B_poT", 2)))
                ppk = ph1.enter_context(tc.tile_pool(name="ppk", bufs=2))
                ps_mm = ph1.enter_context(
                    tc.tile_pool(name="ps_mm", bufs=knob("B_psmm", 5), space="PSUM")
                )
                ps_tr = ph1.enter_context(
                    tc.tile_pool(name="ps_tr", bufs=knob("B_pstr", 1), space="PSUM")
                )
                ps_pk = ph1.enter_context(
                    tc.tile_pool(name="ps_pk", bufs=1, space="PSUM")
                )
                ps_ptr = ph1.enter_context(
                    tc.tile_pool(name="ps_ptr", bufs=1, space="PSUM")
                )

                for ck, (tb, ntok) in enumerate(CHUNKS):
                    x0 = px.tile([128, NKT // 2, ntok], mdt, tag="x0")
                    x1 = px.tile([128, NKT // 2, ntok], mdt, tag="x1")
                    nc.sync.dma_start(x0[:], xt[:, ck, 0 : NKT // 2, :])
                    nc.sync.dma_start(x1[:], xt[:, ck, NKT // 2 : NKT, :])
                    for wb in range(NWB):
                        w_sb = pw.tile([128, NKT, WB_COLS], mdt, tag="w")
                        nc.sync.dma_start(w_sb[:], wt[:, wb])
                        if ck == 0 and wb >= 2:
                            # prefix projections off the same weight stream
                            psp = ps_pk.tile([PFX, WB_COLS], f32, tag="ppk")
                            for ki in range(NKT):
                                nc.tensor.matmul(
                                    psp[:],
                                    lhsT=pf_sb[:, ki, :],
                                    rhs=w_sb[:, ki, :],
                                    start=(ki == 0),
                                    stop=(ki == NKT - 1),
                                )
                            if wb < 4:  # k-cols -> pkT (transposed per head)
                                pks = ppk.tile([PFX, WB_COLS], mdt, tag="pks")
                                nc.scalar.activation(pks[:], psp[:], AF.Copy)
                                for c in range(4):
                                    h = (wb - 2) * 4 + c
                                    ptr = ps_ptr.tile([128, PFX], mdt, tag="ptr")
                                    nc.tensor.transpose(
                                        ptr[:],
                                        pks[:, c * 128 : (c + 1) * 128],
                                        eye_sb[0:PFX, 0:PFX],
                                    )
                                    nc.vector.tensor_copy(pkT_sb[:, h, :], ptr[:])
                            else:  # v-cols -> straight prefix-v
                                nc.scalar.activation(
                                    pv_sb[:, wb - 4, :], psp[:], AF.Copy
                                )
                        if wb < 4:
                            oTa = poT.tile([128, 4, ntok], mdt, tag="oTa")
                        for mt in range(ntok // 128):
                            ps = ps_mm.tile([128, WB_COLS], f32, tag="mm")
                            for ki in range(NKT):
                                xs = x0 if ki < NKT // 2 else x1
                                nc.tensor.matmul(
                                    ps[:],
                                    lhsT=xs[
                                        :, ki % (NKT // 2), mt * 128 : (mt + 1) * 128
                                    ],
                                    rhs=w_sb[:, ki, :],
                                    start=(ki == 0),
                                    stop=(ki == NKT - 1),
                                )
                            kb = ck * 8 + mt
                            if wb < 4:  # q/k: rope on contiguous free-dim slices
                                m1 = ptmp.tile([128, WB_COLS], f32, tag="m1")
                                m2 = ptmp.tile([128, WB_COLS], f32, tag="m2")
                                nc.vector.tensor_tensor(
                                    m1[:], ps[:], s1_sb[:, kb, :], OP.mult
                                )
                                nc.vector.tensor_tensor(
                                    m2[:], ps[:], s2_sb[:, kb, :], OP.mult
                                )
                                o = po.tile([128, WB_COLS], mdt, tag="o")
                                o3 = o[:].rearrange(
                                    "p (hh half f) -> p hh half f", half=2, f=64
                                )
                                m1r = m1[:].rearrange(
                                    "p (hh half f) -> p hh half f", half=2, f=64
                                )
                                m2r = m2[:].rearrange(
                                    "p (hh half f) -> p hh half f", half=2, f=64
                                )
                                nc.vector.tensor_tensor(
                                    o3[:, :, 0, :],
                                    m1r[:, :, 0, :],
                                    m1r[:, :, 1, :],
                                    OP.subtract,
                                )
                                nc.vector.tensor_tensor(
                                    o3[:, :, 1, :],
                                    m2r[:, :, 0, :],
                                    m2r[:, :, 1, :],
                                    OP.add,
                                )
                                for c in range(4):
                                    ptr2 = ps_tr.tile([128, 128], mdt, tag="tr")
                                    nc.tensor.transpose(
                                        ptr2[:],
                                        o[:, c * 128 : (c + 1) * 128],
                                        eye_sb[:],
                                    )
                                    nc.scalar.activation(
                                        oTa[:, c, mt * 128 : (mt + 1) * 128],
                                        ptr2[:],
                                        AF.Copy,
                                    )
                            else:  # v: straight copy out
                                o = po.tile([128, WB_COLS], mdt, tag="o")
                                nc.scalar.activation(o[:], ps[:], AF.Copy)
                                col0 = (wb - 4) * WB_COLS
                                nc.scalar.dma_start(
                                    v_sp[:, kb, col0 : col0 + WB_COLS], o[:]
                                )
                        if wb < 4:  # spill accumulated [128, ntok] rows per col-tile
                            for c in range(4):
                                row0 = wb * WB_COLS + c * 128
                                nc.scalar.dma_start(
                                    qkT_sp[row0 : row0 + 128, tb : tb + ntok],
                                    oTa[:, c, :],
                                )

            if not os.environ.get("NOBAR12"):
                tc.strict_bb_all_engine_barrier()
            # ---------------- Phase 2: attention ----------------
            with ExitStack() as ph2:
              if True:
                pwo = ph2.enter_context(tc.tile_pool(name="pwo", bufs=1))
                # attention output, SBUF-resident across phases 2+3
                patt = ph2.enter_context(tc.tile_pool(name="patt", bufs=1))
                att_res = patt.tile([128, HPC, S], mdt)
                with ExitStack() as ph2i:
                  if "2" in phases:
                    wo_sb = pwo.tile([128, COLS // 128, D], mdt, tag="wo")
                    nc.sync.dma_start(wo_sb[:], wot[:])
                    pmask = ph2i.enter_context(tc.tile_pool(name="pmask", bufs=1))
                    masks_sb = pmask.tile([128, 4, 512], mdt)
                    nc.sync.dma_start(masks_sb[:], masks[:])
                    pkv = ph2i.enter_context(tc.tile_pool(name="pkv", bufs=knob("B_pkv", 2)))
                    pva = ph2i.enter_context(tc.tile_pool(name="pva", bufs=1))
                    pq = ph2i.enter_context(tc.tile_pool(name="pq", bufs=2))
                    pE = ph2i.enter_context(tc.tile_pool(name="pE", bufs=knob("B_pE", 4)))
                    pc = ph2i.enter_context(tc.tile_pool(name="pc", bufs=2))
                    ps_s = ph2i.enter_context(
                        tc.tile_pool(name="ps_s", bufs=knob("B_pss", 2), space="PSUM")
                    )
                    ps_pv = ph2i.enter_context(
                        tc.tile_pool(name="ps_pv", bufs=2, space="PSUM")
                    )
                    ps_den = ph2i.enter_context(
                        tc.tile_pool(name="ps_den", bufs=1, space="PSUM")
                    )
                    ps_pfx = ph2i.enter_context(
                        tc.tile_pool(name="ps_pfx", bufs=1, space="PSUM")
                    )

                    # all of v resident: [tok%128, tok//128, col]
                    vv = pva.tile([128, S // 128, COLS], mdt, tag="v")
                    nc.sync.dma_start(vv[:], v_sp[:])
                    for h in range(HPC):
                        kT = pkv.tile([128, S], mdt, tag="kT")
                        nc.sync.dma_start(
                            kT[:], qkT_sp[COLS + h * 128 : COLS + (h + 1) * 128, :]
                        )
                        q_h = pq.tile([128, S], mdt, tag="q")
                        nc.sync.dma_start(
                            q_h[:], qkT_sp[h * 128 : (h + 1) * 128, :]
                        )
                        for qb in range(4):
                            q_sb = q_h[:, qb * 512 : (qb + 1) * 512]
                            nkb = 4 * qb + 4
                            pv_ps = ps_pv.tile([128, 512], f32, tag="pv")
                            den_ps = ps_den.tile([1, 512], f32, tag="den")
                            for kb in range(nkb):
                                s_ps = ps_s.tile([128, 512], f32, tag="s")
                                nc.tensor.matmul(
                                    s_ps[:],
                                    lhsT=kT[:, kb * 128 : (kb + 1) * 128],
                                    rhs=q_sb,
                                    start=True,
                                    stop=True,
                                )
                                E = pE.tile([128, 512], mdt, tag="E")
                                nc.scalar.activation(
                                    E[:], s_ps[:], AF.Exp, scale=SCALE
                                )
                                t = kb - 4 * qb
                                if t >= 0:
                                    nc.vector.tensor_tensor(
                                        E[:], E[:], masks_sb[:, t, :], OP.mult
                                    )
                                nc.tensor.matmul(
                                    pv_ps[:],
                                    lhsT=vv[:, kb, h * 128 : (h + 1) * 128],
                                    rhs=E[:],
                                    start=(kb == 0),
                                    stop=(kb == nkb - 1),
                                )
                                nc.tensor.matmul(
                                    den_ps[:],
                                    lhsT=ones_sb[:],
                                    rhs=E[:],
                                    start=(kb == 0),
                                    stop=(kb == nkb - 1),
                                )
                            # prefix branch
                            sp_ps = ps_pfx.tile([PFX, 512], f32, tag="sp")
                            nc.tensor.matmul(
                                sp_ps[:],
                                lhsT=pkT_sb[:, h, :],
                                rhs=q_sb,
                                start=True,
                                stop=True,
                            )
                            EP = pE.tile([PFX, 512], mdt, tag="EP")
                            nc.scalar.activation(EP[:], sp_ps[:], AF.Exp, scale=SCALE)
                            pvP_ps = ps_pfx.tile([128, 512], f32, tag="pvP")
                            nc.tensor.matmul(
                                pvP_ps[:],
                                lhsT=pv_sb[
                                    :, h // 4, (h % 4) * 128 : (h % 4) * 128 + 128
                                ],
                                rhs=EP[:],
                                start=True,
                                stop=True,
                            )
                            denP_ps = ps_pfx.tile([1, 512], f32, tag="denP")
                            nc.tensor.matmul(
                                denP_ps[:],
                                lhsT=ones_sb[0:PFX, :],
                                rhs=EP[:],
                                start=True,
                                stop=True,
                            )
                            # combine: att = pv/den + g * pvP/denP
                            r1 = pc.tile([1, 512], f32, tag="r1")
                            nc.vector.reciprocal(r1[:], den_ps[:])
                            r2 = pc.tile([1, 512], f32, tag="r2")
                            nc.vector.reciprocal(r2[:], denP_ps[:])
                            nc.vector.tensor_scalar_mul(
                                r2[:], r2[:], g_sb[0:1, h : h + 1]
                            )
                            rb1 = pc.tile([128, 512], f32, tag="rb1")
                            nc.gpsimd.partition_broadcast(rb1[:], r1[:])
                            rb2 = pc.tile([128, 512], f32, tag="rb2")
                            nc.gpsimd.partition_broadcast(rb2[:], r2[:])
                            t1 = pc.tile([128, 512], f32, tag="t1")
                            nc.vector.tensor_tensor(t1[:], pv_ps[:], rb1[:], OP.mult)
                            t2 = pc.tile([128, 512], f32, tag="t2")
                            nc.vector.tensor_tensor(t2[:], pvP_ps[:], rb2[:], OP.mult)
                            nc.vector.tensor_tensor(
                                att_res[:, h, qb * 512 : (qb + 1) * 512],
                                t1[:],
                                t2[:],
                                OP.add,
                            )

                if not os.environ.get("NOBAR23"):
                    tc.strict_bb_all_engine_barrier()
                # ---------------- Phase 3: output projection ----------------
                with ExitStack() as ph3:
                  if "3" in phases:
                    pacc = ph3.enter_context(tc.tile_pool(name="pacc", bufs=2))
                    ps3 = ph3.enter_context(
                        tc.tile_pool(name="ps3", bufs=knob("B_ps3", 8), space="PSUM")
                    )
                    if "2" not in phases:  # wo not loaded by phase 2 slice
                        wo_sb = pwo.tile([128, COLS // 128, D], mdt, tag="wo")
                        nc.sync.dma_start(wo_sb[:], wot[:])
                    for mt in range(S // 128):
                        o_acc = pacc.tile([128, D], f32, tag="oacc")
                        for nb in range(D // 512):
                            ps = ps3.tile([128, 512], f32, tag="mm3")
                            for kc in range(COLS // 128):
                                nc.tensor.matmul(
                                    ps[:],
                                    lhsT=att_res[:, kc, mt * 128 : (mt + 1) * 128],
                                    rhs=wo_sb[:, kc, nb * 512 : (nb + 1) * 512],
                                    start=(kc == 0),
                                    stop=(kc == COLS // 128 - 1),
                                )
                            nc.scalar.activation(
                                o_acc[:, nb * 512 : (nb + 1) * 512], ps[:], AF.Copy
                            )
                        nc.sync.dma_start(
                            out_d[mt * 128 : (mt + 1) * 128, :], o_acc[:]
                        )

    nc.compile()
    return nc


# even/odd-split permutation within each head's 128 q/k columns
_PERM = np.concatenate([np.arange(0, HD, 2), np.arange(1, HD, 2)])


def _host_inputs(x, freqs_cos, freqs_sin, prefix, prefix_gate, wq, wk, wv, wo):
    import ml_dtypes

    bf16 = ml_dtypes.bfloat16
    x = np.asarray(x, np.float32)
    freqs_cos = np.asarray(freqs_cos, np.float32)
    freqs_sin = np.asarray(freqs_sin, np.float32)
    prefix = np.asarray(prefix, np.float32)
    prefix_gate = np.asarray(prefix_gate, np.float32)
    wq = np.asarray(wq, np.float32)
    wk = np.asarray(wk, np.float32)
    wv = np.asarray(wv, np.float32)
    wo = np.asarray(wo, np.float32)

    # permute q/k columns to even/odd-split order within each head
    colperm = (np.arange(H)[:, None] * HD + _PERM[None, :]).reshape(-1)
    wq = wq[:, colperm]
    wk = wk[:, colperm]

    # rope factor tables [tok%128, tok//128, 512-col pattern] (4 heads/block)
    a_small = np.concatenate([freqs_cos, freqs_sin], 1)  # [S, 128]
    b_small = np.concatenate([freqs_sin, freqs_cos], 1)
    s1 = np.ascontiguousarray(
        np.tile(a_small, (1, 4)).reshape(S // 128, 128, WB_COLS).transpose(1, 0, 2)
    ).astype(bf16)
    s2 = np.ascontiguousarray(
        np.tile(b_small, (1, 4)).reshape(S // 128, 128, WB_COLS).transpose(1, 0, 2)
    ).astype(bf16)

    ii = np.arange(128)[:, None, None]
    tt_ = np.arange(4)[None, :, None]
    jj = np.arange(512)[None, None, :]
    masks = (jj >= ii + 128 * tt_).astype(bf16)
    ones = np.ones((128, 1), bf16)
    eye = np.eye(128, dtype=bf16)
    pft = np.ascontiguousarray(
        prefix[0].T.reshape(NKT, 128, PFX).transpose(1, 0, 2)
    ).astype(bf16)
    g = np.tanh(prefix_gate)

    xts = [
        np.ascontiguousarray(
            x[b].T.reshape(NKT, 128, 2, CHUNK).transpose(1, 2, 0, 3)
        ).astype(bf16)
        for b in range(B)
    ]
    in_maps = []
    for c in range(NCORES):
        b, gi = divmod(c, CPB)
        cols = slice(gi * COLS, (gi + 1) * COLS)
        wqkv = np.concatenate([wq[:, cols], wk[:, cols], wv[:, cols]], axis=1)
        wt = np.ascontiguousarray(
            wqkv.reshape(NKT, 128, NWB, WB_COLS).transpose(1, 2, 0, 3)
        ).astype(bf16)
        wot = np.ascontiguousarray(
            wo[cols, :].reshape(COLS // 128, 128, D).transpose(1, 0, 2)
        ).astype(bf16)
        in_maps.append(
            dict(
                xt=xts[b],
                wt=wt,
                wot=wot,
                pft=pft,
                s1=s1,
                s2=s2,
                masks=masks,
                ones=ones,
                eye=eye,
                g=np.ascontiguousarray(g[None, gi * HPC : (gi + 1) * HPC]).astype(
                    np.float32
                ),
            )
        )
    return in_maps


def _run(inputs, trace=False, mm_fp32r=True):
    from concourse.bass_utils import run_bass_kernel_spmd

    key = ("nc", mm_fp32r)
    if key not in _CACHE:
        _CACHE[key] = _build(mm_fp32r)
    nc = _CACHE[key]
    in_maps = _host_inputs(
        inputs["x"],
        inputs["freqs_cos"],
        inputs["freqs_sin"],
        inputs["prefix"],
        inputs["prefix_gate"],
        inputs["wq"],
        inputs["wk"],
        inputs["wv"],
        inputs["wo"],
    )
    res = run_bass_kernel_spmd(nc, in_maps, list(range(NCORES)), trace=trace)
    parts = [res.results[c]["out"] for c in range(NCORES)]
    out = np.stack(
        [
            parts[0] + parts[1] + parts[2] + parts[3],
            parts[4] + parts[5] + parts[6] + parts[7],
        ],
        axis=0,
    ).astype(np.float32)
    return out, res


def kernel(**inputs) -> np.ndarray:
    out, _ = _run(inputs, trace=False)
    return out


# revision 41
# speedup vs baseline: 1.0785x; 1.0785x over previous
"""Trainium2 Bass kernel for nn_Attention_46901042872408.

Dense MHA transformer block with RoPE + prefix-tuning branch:
  q/k/v = x @ wq/wk/wv; rope(q), rope(k); causal attention;
  prefix branch: non-causal attention of q against (prefix @ wk/wv),
  gated by tanh(prefix_gate) per head; out = (attn + gate*prefix_attn) @ wo.

Sharding: 8 cores = data-parallel over batch (2) x tensor-parallel over
heads (4 groups of 8 heads). Each core computes a partial [2048, 4096]
output (its heads' contribution through its wo row-slice); host sums the
4 partials per batch.

All matmul operands bf16 (PSUM accumulates f32); tolerance is 2e-2 so
bf16's ~1e-3 matmul error is fine and it halves DMA + SBUF.

Key structure (instruction/DMA count minimized; ~0.8ms of the chained
measurement is fixed per-exec dispatch overhead that hides most of
phase 1, so phases 2+3 are the exposed cost):
  Phase 1: x-stationary projections over 2 chunks of 1024 tokens,
    streaming host-pre-tiled 512-col weight blocks (contiguous 32KB
    per-partition DMA runs). wq/wk columns host-permuted to
    even/odd-split order within each head so RoPE is 4 contiguous
    free-dim DVE ops against resident factor tables; q/k PE-transposed
    to [hd, token], accumulated to [128, 1024] tiles, spilled in 2KB
    rows; v spilled straight in [tok%128, tok//128, col] layout.
    Prefix k/v projections ride the same weight stream.
  Phase 2: v loaded once for all heads (one contiguous DMA); per head:
    kT + q loaded once; per (head, 512-token q-block): scores^T [k,q]
    tiles, exp on ACT (bf16 out), causal mask multiply on DVE only for
    diagonal tiles, PV + ones-denominator matmuls accumulated in PSUM,
    prefix branch with 30 k-rows, combine via reciprocal + gpsimd
    partition-broadcast; attention output stays SBUF-resident.
  Phase 3: out = attnT.T @ wo with wo SBUF-resident (loaded during
    phase 2), accumulated to [128, 4096] f32 row-blocks, 16 writes.

Rejected via measurement: gpsimd C-reduce for the softmax denominator
(65us/tile on HW, 13x regression), DMA-transpose loads + Pool-queue DMA
issue + Pool mask-multiplies (+11%), fp8 (error budget), PSUM-direct
DRAM writes (unsupported).
"""

import sys

sys.path.insert(0, "/opt/trn_rl_repo")

import numpy as np

B, S, D = 2, 2048, 4096
H, HD = 32, 128
PFX = 30
NCORES = 8
CPB = 4  # cores per batch (head-parallel groups)
HPC = 8  # heads per core
COLS = HPC * HD  # 1024 qkv columns per core
WB_COLS = 512  # weight column-block
NWB = 3 * COLS // WB_COLS  # 6 weight blocks: 0-1 q, 2-3 k, 4-5 v
NKT = D // 128  # 32 contraction tiles
CHUNK = 1024
CHUNKS = [(0, CHUNK), (CHUNK, CHUNK)]
SCALE = 1.0 / float(np.sqrt(HD))

_CACHE = {}


def _build(mm_fp32r=True):
    import os
    from contextlib import ExitStack

    phases = os.environ.get("KPHASES", "123")

    def knob(name, default):
        return int(os.environ.get(name, default))

    import concourse.tile as tile
    from concourse import bacc, mybir

    f32 = mybir.dt.float32
    mdt = mybir.dt.bfloat16
    AF = mybir.ActivationFunctionType
    OP = mybir.AluOpType

    nc = bacc.Bacc("TRN2", target_bir_lowering=False, debug=False, num_devices=NCORES)

    # host-pre-tiled inputs (contiguous per-partition DMA runs)
    xt = nc.dram_tensor("xt", [128, 2, NKT, CHUNK], mdt, kind="ExternalInput")
    wt = nc.dram_tensor("wt", [128, NWB, NKT, WB_COLS], mdt, kind="ExternalInput")
    wot = nc.dram_tensor("wot", [128, COLS // 128, D], mdt, kind="ExternalInput")
    pft = nc.dram_tensor("pft", [128, NKT, PFX], mdt, kind="ExternalInput")
    # rope factor tables in [tok%128, tok//128, col-pattern] layout
    s1_d = nc.dram_tensor("s1", [128, S // 128, WB_COLS], mdt, kind="ExternalInput")
    s2_d = nc.dram_tensor("s2", [128, S // 128, WB_COLS], mdt, kind="ExternalInput")
    masks = nc.dram_tensor("masks", [128, 4, 512], mdt, kind="ExternalInput")
    ones_d = nc.dram_tensor("ones", [128, 1], mdt, kind="ExternalInput")
    eye_d = nc.dram_tensor("eye", [128, 128], mdt, kind="ExternalInput")
    g_d = nc.dram_tensor("g", [1, HPC], f32, kind="ExternalInput")
    out_d = nc.dram_tensor("out", [S, D], f32, kind="ExternalOutput")

    with tile.TileContext(nc) as tc:
        with ExitStack() as top:
            dram = top.enter_context(tc.tile_pool(name="dram", bufs=1, space="DRAM"))
            # per-weight-block spill tiles -> finer cross-phase deps
            qk_sp0 = dram.tile([WB_COLS, S], mdt, tag="qk0")
            qk_sp1 = dram.tile([WB_COLS, S], mdt, tag="qk1")
            qk_sp2 = dram.tile([WB_COLS, S], mdt, tag="qk2")
            qk_sp3 = dram.tile([WB_COLS, S], mdt, tag="qk3")
            qkT_sp = [qk_sp0, qk_sp1, qk_sp2, qk_sp3]
            v_sp = dram.tile([128, S // 128, COLS], mdt)  # [tok%128, tok//128, col]

            pres = top.enter_context(tc.tile_pool(name="res", bufs=1))
            pf_sb = pres.tile([128, NKT, PFX], mdt)
            nc.sync.dma_start(pf_sb[:], pft[:])
            eye_sb = pres.tile([128, 128], mdt)
            nc.sync.dma_start(eye_sb[:], eye_d[:])
            ones_sb = pres.tile([128, 1], mdt)
            nc.sync.dma_start(ones_sb[:], ones_d[:])
            g_sb = pres.tile([1, HPC], f32)
            nc.sync.dma_start(g_sb[:], g_d[:])
            pkT_sb = pres.tile([128, HPC, PFX], mdt)
            pv_sb = pres.tile([PFX, 2, WB_COLS], mdt)  # straight prefix-v, 4 heads/blk

            # ---------------- Phase 1: projections ----------------
            with ExitStack() as ph1:
              if "1" in phases:
                pcs = ph1.enter_context(tc.tile_pool(name="pcs", bufs=1))
                s1_sb = pcs.tile([128, S // 128, WB_COLS], mdt)
                nc.sync.dma_start(s1_sb[:], s1_d[:])
                s2_sb = pcs.tile([128, S // 128, WB_COLS], mdt)
                nc.sync.dma_start(s2_sb[:], s2_d[:])
                px = ph1.enter_context(tc.tile_pool(name="px", bufs=1))
                pw = ph1.enter_context(tc.tile_pool(name="pw", bufs=knob("B_pw", 2)))
                ptmp = ph1.enter_context(tc.tile_pool(name="ptmp", bufs=knob("B_ptmp", 2)))
                po = ph1.enter_context(tc.tile_pool(name="po", bufs=knob("B_po", 3)))
                poT = ph1.enter_context(tc.tile_pool(name="poT", bufs=knob("B_poT", 2)))
                ppk = ph1.enter_context(tc.tile_pool(name="ppk", bufs=2))
                ps_mm = ph1.enter_context(
                    tc.tile_pool(name="ps_mm", bufs=knob("B_psmm", 5), space="PSUM")
                )
                ps_tr = ph1.enter_context(
                    tc.tile_pool(name="ps_tr", bufs=knob("B_pstr", 1), space="PSUM")
                )
                ps_pk = ph1.enter_context(
                    tc.tile_pool(name="ps_pk", bufs=1, space="PSUM")
                )
                ps_ptr = ph1.enter_context(
                    tc.tile_pool(name="ps_ptr", bufs=1, space="PSUM")
                )

                for ck, (tb, ntok) in enumerate(CHUNKS):
                    x0 = px.tile([128, NKT // 2, ntok], mdt, tag="x0")
                    x1 = px.tile([128, NKT // 2, ntok], mdt, tag="x1")
                    nc.sync.dma_start(x0[:], xt[:, ck, 0 : NKT // 2, :])
                    nc.sync.dma_start(x1[:], xt[:, ck, NKT // 2 : NKT, :])
                    for wb in (4, 5, 2, 3, 0, 1):
                        w_sb = pw.tile([128, NKT, WB_COLS], mdt, tag="w")
                        nc.sync.dma_start(w_sb[:], wt[:, wb])
                        if ck == 0 and wb >= 2:
                            # prefix projections off the same weight stream
                            psp = ps_pk.tile([PFX, WB_COLS], f32, tag="ppk")
                            for ki in range(NKT):
                                nc.tensor.matmul(
                                    psp[:],
                                    lhsT=pf_sb[:, ki, :],
                                    rhs=w_sb[:, ki, :],
                                    start=(ki == 0),
                                    stop=(ki == NKT - 1),
                                )
                            if wb < 4:  # k-cols -> pkT (transposed per head)
                                pks = ppk.tile([PFX, WB_COLS], mdt, tag="pks")
                                nc.scalar.activation(pks[:], psp[:], AF.Copy)
                                for c in range(4):
                                    h = (wb - 2) * 4 + c
                                    ptr = ps_ptr.tile([128, PFX], mdt, tag="ptr")
                                    nc.tensor.transpose(
                                        ptr[:],
                                        pks[:, c * 128 : (c + 1) * 128],
                                        eye_sb[0:PFX, 0:PFX],
                                    )
                                    nc.vector.tensor_copy(pkT_sb[:, h, :], ptr[:])
                            else:  # v-cols -> straight prefix-v
                                nc.scalar.activation(
                                    pv_sb[:, wb - 4, :], psp[:], AF.Copy
                                )
                        if wb < 4:
                            oTa = poT.tile([128, 4, ntok], mdt, tag="oTa")
                        for mt in range(ntok // 128):
                            ps = ps_mm.tile([128, WB_COLS], f32, tag="mm")
                            for ki in range(NKT):
                                xs = x0 if ki < NKT // 2 else x1
                                nc.tensor.matmul(
                                    ps[:],
                                    lhsT=xs[
                                        :, ki % (NKT // 2), mt * 128 : (mt + 1) * 128
                                    ],
                                    rhs=w_sb[:, ki, :],
                                    start=(ki == 0),
                                    stop=(ki == NKT - 1),
                                )
                            kb = ck * 8 + mt
                            if wb < 4:  # q/k: rope on contiguous free-dim slices
                                m1 = ptmp.tile([128, WB_COLS], f32, tag="m1")
                                m2 = ptmp.tile([128, WB_COLS], f32, tag="m2")
                                nc.vector.tensor_tensor(
                                    m1[:], ps[:], s1_sb[:, kb, :], OP.mult
                                )
                                nc.vector.tensor_tensor(
                                    m2[:], ps[:], s2_sb[:, kb, :], OP.mult
                                )
                                o = po.tile([128, WB_COLS], mdt, tag="o")
                                o3 = o[:].rearrange(
                                    "p (hh half f) -> p hh half f", half=2, f=64
                                )
                                m1r = m1[:].rearrange(
                                    "p (hh half f) -> p hh half f", half=2, f=64
                                )
                                m2r = m2[:].rearrange(
                                    "p (hh half f) -> p hh half f", half=2, f=64
                                )
                                nc.vector.tensor_tensor(
                                    o3[:, :, 0, :],
                                    m1r[:, :, 0, :],
                                    m1r[:, :, 1, :],
                                    OP.subtract,
                                )
                                nc.vector.tensor_tensor(
                                    o3[:, :, 1, :],
                                    m2r[:, :, 0, :],
                                    m2r[:, :, 1, :],
                                    OP.add,
                                )
                                for c in range(4):
                                    ptr2 = ps_tr.tile([128, 128], mdt, tag="tr")
                                    nc.tensor.transpose(
                                        ptr2[:],
                                        o[:, c * 128 : (c + 1) * 128],
                                        eye_sb[:],
                                    )
                                    nc.scalar.activation(
                                        oTa[:, c, mt * 128 : (mt + 1) * 128],
                                        ptr2[:],
                                        AF.Copy,
                                    )
                            else:  # v: straight copy out
                                o = po.tile([128, WB_COLS], mdt, tag="o")
                                nc.scalar.activation(o[:], ps[:], AF.Copy)
                                col0 = (wb - 4) * WB_COLS
                                nc.scalar.dma_start(
                                    v_sp[:, kb, col0 : col0 + WB_COLS], o[:]
                                )
                        if wb < 4:  # spill accumulated [128, ntok] rows per col-tile
                            for c in range(4):
                                nc.scalar.dma_start(
                                    qkT_sp[wb][c * 128 : (c + 1) * 128, tb : tb + ntok],
                                    oTa[:, c, :],
                                )

            if os.environ.get("FORCEBAR"):
                tc.strict_bb_all_engine_barrier()
            # ---------------- Phase 2: attention ----------------
            with ExitStack() as ph2:
              if True:
                pwo = ph2.enter_context(tc.tile_pool(name="pwo", bufs=1))
                # attention output, SBUF-resident across phases 2+3
                patt = ph2.enter_context(tc.tile_pool(name="patt", bufs=1))
                att_res = patt.tile([128, HPC, S], mdt)
                with ExitStack() as ph2i:
                  if "2" in phases:
                    wo_sb = pwo.tile([128, COLS // 128, D], mdt, tag="wo")
                    nc.sync.dma_start(wo_sb[:], wot[:])
                    pmask = ph2i.enter_context(tc.tile_pool(name="pmask", bufs=1))
                    masks_sb = pmask.tile([128, 4, 512], mdt)
                    nc.sync.dma_start(masks_sb[:], masks[:])
                    pkv = ph2i.enter_context(tc.tile_pool(name="pkv", bufs=knob("B_pkv", 2)))
                    pva = ph2i.enter_context(tc.tile_pool(name="pva", bufs=1))
                    pq = ph2i.enter_context(tc.tile_pool(name="pq", bufs=2))
                    pE = ph2i.enter_context(tc.tile_pool(name="pE", bufs=knob("B_pE", 4)))
                    pc = ph2i.enter_context(tc.tile_pool(name="pc", bufs=2))
                    ps_s = ph2i.enter_context(
                        tc.tile_pool(name="ps_s", bufs=knob("B_pss", 2), space="PSUM")
                    )
                    ps_pv = ph2i.enter_context(
                        tc.tile_pool(name="ps_pv", bufs=2, space="PSUM")
                    )
                    ps_den = ph2i.enter_context(
                        tc.tile_pool(name="ps_den", bufs=1, space="PSUM")
                    )
                    ps_pfx = ph2i.enter_context(
                        tc.tile_pool(name="ps_pfx", bufs=1, space="PSUM")
                    )

                    # all of v resident: [tok%128, tok//128, col]
                    vv = pva.tile([128, S // 128, COLS], mdt, tag="v")
                    nc.sync.dma_start(vv[:], v_sp[:])
                    for h in range(HPC):
                        kT = pkv.tile([128, S], mdt, tag="kT")
                        nc.sync.dma_start(
                            kT[:],
                            qkT_sp[2 + h // 4][(h % 4) * 128 : (h % 4) * 128 + 128, :],
                        )
                        q_h = pq.tile([128, S], mdt, tag="q")
                        nc.sync.dma_start(
                            q_h[:],
                            qkT_sp[h // 4][(h % 4) * 128 : (h % 4) * 128 + 128, :],
                        )
                        for qb in range(4):
                            q_sb = q_h[:, qb * 512 : (qb + 1) * 512]
                            nkb = 4 * qb + 4
                            pv_ps = ps_pv.tile([128, 512], f32, tag="pv")
                            den_ps = ps_den.tile([1, 512], f32, tag="den")
                            for kb in range(nkb):
                                s_ps = ps_s.tile([128, 512], f32, tag="s")
                                nc.tensor.matmul(
                                    s_ps[:],
                                    lhsT=kT[:, kb * 128 : (kb + 1) * 128],
                                    rhs=q_sb,
                                    start=True,
                                    stop=True,
                                )
                                E = pE.tile([128, 512], mdt, tag="E")
                                nc.scalar.activation(
                                    E[:], s_ps[:], AF.Exp, scale=SCALE
                                )
                                t = kb - 4 * qb
                                if t >= 0:
                                    nc.vector.tensor_tensor(
                                        E[:], E[:], masks_sb[:, t, :], OP.mult
                                    )
                                nc.tensor.matmul(
                                    pv_ps[:],
                                    lhsT=vv[:, kb, h * 128 : (h + 1) * 128],
                                    rhs=E[:],
                                    start=(kb == 0),
                                    stop=(kb == nkb - 1),
                                )
                                nc.tensor.matmul(
                                    den_ps[:],
                                    lhsT=ones_sb[:],
                                    rhs=E[:],
                                    start=(kb == 0),
                                    stop=(kb == nkb - 1),
                                )
                            # prefix branch
                            sp_ps = ps_pfx.tile([PFX, 512], f32, tag="sp")
                            nc.tensor.matmul(
                                sp_ps[:],
                                lhsT=pkT_sb[:, h, :],
                                rhs=q_sb,
                                start=True,
                                stop=True,
                            )
                            EP = pE.tile([PFX, 512], mdt, tag="EP")
                            nc.scalar.activation(EP[:], sp_ps[:], AF.Exp, scale=SCALE)
                            pvP_ps = ps_pfx.tile([128, 512], f32, tag="pvP")
                            nc.tensor.matmul(
                                pvP_ps[:],
                                lhsT=pv_sb[
                                    :, h // 4, (h % 4) * 128 : (h % 4) * 128 + 128
                                ],
                                rhs=EP[:],
                                start=True,
                                stop=True,
                            )
                            denP_ps = ps_pfx.tile([1, 512], f32, tag="denP")
                            nc.tensor.matmul(
                                denP_ps[:],
                                lhsT=ones_sb[0:PFX, :],
                                rhs=EP[:],
                                start=True,
                                stop=True,
                            )
                            # combine: att = pv/den + g * pvP/denP
                            r1 = pc.tile([1, 512], f32, tag="r1")
                            nc.vector.reciprocal(r1[:], den_ps[:])
                            r2 = pc.tile([1, 512], f32, tag="r2")
                            nc.vector.reciprocal(r2[:], denP_ps[:])
                            nc.vector.tensor_scalar_mul(
                                r2[:], r2[:], g_sb[0:1, h : h + 1]
                            )
                            rb1 = pc.tile([128, 512], f32, tag="rb1")
                            nc.gpsimd.partition_broadcast(rb1[:], r1[:])
                            rb2 = pc.tile([128, 512], f32, tag="rb2")
                            nc.gpsimd.partition_broadcast(rb2[:], r2[:])
                            t1 = pc.tile([128, 512], f32, tag="t1")
                            nc.vector.tensor_tensor(t1[:], pv_ps[:], rb1[:], OP.mult)
                            t2 = pc.tile([128, 512], f32, tag="t2")
                            nc.vector.tensor_tensor(t2[:], pvP_ps[:], rb2[:], OP.mult)
                            nc.vector.tensor_tensor(
                                att_res[:, h, qb * 512 : (qb + 1) * 512],
                                t1[:],
                                t2[:],
                                OP.add,
                            )

                if os.environ.get("FORCEBAR"):
                    tc.strict_bb_all_engine_barrier()
                # ---------------- Phase 3: output projection ----------------
                with ExitStack() as ph3:
                  if "3" in phases:
                    pacc = ph3.enter_context(tc.tile_pool(name="pacc", bufs=2))
                    ps3 = ph3.enter_context(
                        tc.tile_pool(name="ps3", bufs=knob("B_ps3", 8), space="PSUM")
                    )
                    if "2" not in phases:  # wo not loaded by phase 2 slice
                        wo_sb = pwo.tile([128, COLS // 128, D], mdt, tag="wo")
                        nc.sync.dma_start(wo_sb[:], wot[:])
                    for mt in range(S // 128):
                        o_acc = pacc.tile([128, D], f32, tag="oacc")
                        for nb in range(D // 512):
                            ps = ps3.tile([128, 512], f32, tag="mm3")
                            for kc in range(COLS // 128):
                                nc.tensor.matmul(
                                    ps[:],
                                    lhsT=att_res[:, kc, mt * 128 : (mt + 1) * 128],
                                    rhs=wo_sb[:, kc, nb * 512 : (nb + 1) * 512],
                                    start=(kc == 0),
                                    stop=(kc == COLS // 128 - 1),
                                )
                            nc.scalar.activation(
                                o_acc[:, nb * 512 : (nb + 1) * 512], ps[:], AF.Copy
                            )
                        nc.sync.dma_start(
                            out_d[mt * 128 : (mt + 1) * 128, :], o_acc[:]
                        )

    nc.compile()
    return nc


# even/odd-split permutation within each head's 128 q/k columns
_PERM = np.concatenate([np.arange(0, HD, 2), np.arange(1, HD, 2)])


def _host_inputs(x, freqs_cos, freqs_sin, prefix, prefix_gate, wq, wk, wv, wo):
    import ml_dtypes

    bf16 = ml_dtypes.bfloat16
    x = np.asarray(x, np.float32)
    freqs_cos = np.asarray(freqs_cos, np.float32)
    freqs_sin = np.asarray(freqs_sin, np.float32)
    prefix = np.asarray(prefix, np.float32)
    prefix_gate = np.asarray(prefix_gate, np.float32)
    wq = np.asarray(wq, np.float32)
    wk = np.asarray(wk, np.float32)
    wv = np.asarray(wv, np.float32)
    wo = np.asarray(wo, np.float32)

    # permute q/k columns to even/odd-split order within each head
    colperm = (np.arange(H)[:, None] * HD + _PERM[None, :]).reshape(-1)
    wq = wq[:, colperm]
    wk = wk[:, colperm]

    # rope factor tables [tok%128, tok//128, 512-col pattern] (4 heads/block)
    a_small = np.concatenate([freqs_cos, freqs_sin], 1)  # [S, 128]
    b_small = np.concatenate([freqs_sin, freqs_cos], 1)
    s1 = np.ascontiguousarray(
        np.tile(a_small, (1, 4)).reshape(S // 128, 128, WB_COLS).transpose(1, 0, 2)
    ).astype(bf16)
    s2 = np.ascontiguousarray(
        np.tile(b_small, (1, 4)).reshape(S // 128, 128, WB_COLS).transpose(1, 0, 2)
    ).astype(bf16)

    ii = np.arange(128)[:, None, None]
    tt_ = np.arange(4)[None, :, None]
    jj = np.arange(512)[None, None, :]
    masks = (jj >= ii + 128 * tt_).astype(bf16)
    ones = np.ones((128, 1), bf16)
    eye = np.eye(128, dtype=bf16)
    pft = np.ascontiguousarray(
        prefix[0].T.reshape(NKT, 128, PFX).transpose(1, 0, 2)
    ).astype(bf16)
    g = np.tanh(prefix_gate)

    xts = [
        np.ascontiguousarray(
            x[b].T.reshape(NKT, 128, 2, CHUNK).transpose(1, 2, 0, 3)
        ).astype(bf16)
        for b in range(B)
    ]
    in_maps = []
    for c in range(NCORES):
        b, gi = divmod(c, CPB)
        cols = slice(gi * COLS, (gi + 1) * COLS)
        wqkv = np.concatenate([wq[:, cols], wk[:, cols], wv[:, cols]], axis=1)
        wt = np.ascontiguousarray(
            wqkv.reshape(NKT, 128, NWB, WB_COLS).transpose(1, 2, 0, 3)
        ).astype(bf16)
        wot = np.ascontiguousarray(
            wo[cols, :].reshape(COLS // 128, 128, D).transpose(1, 0, 2)
        ).astype(bf16)
        in_maps.append(
            dict(
                xt=xts[b],
                wt=wt,
                wot=wot,
                pft=pft,
                s1=s1,
                s2=s2,
                masks=masks,
                ones=ones,
                eye=eye,
                g=np.ascontiguousarray(g[None, gi * HPC : (gi + 1) * HPC]).astype(
                    np.float32
                ),
            )
        )
    return in_maps


def _run(inputs, trace=False, mm_fp32r=True):
    from concourse.bass_utils import run_bass_kernel_spmd

    key = ("nc", mm_fp32r)
    if key not in _CACHE:
        _CACHE[key] = _build(mm_fp32r)
    nc = _CACHE[key]
    in_maps = _host_inputs(
        inputs["x"],
        inputs["freqs_cos"],
        inputs["freqs_sin"],
        inputs["prefix"],
        inputs["prefix_gate"],
        inputs["wq"],
        inputs["wk"],
        inputs["wv"],
        inputs["wo"],
    )
    res = run_bass_kernel_spmd(nc, in_maps, list(range(NCORES)), trace=trace)
    parts = [res.results[c]["out"] for c in range(NCORES)]
    out = np.stack(
        [
            parts[0] + parts[1] + parts[2] + parts[3],
            parts[4] + parts[5] + parts[6] + parts[7],
        ],
        axis=0,
    ).astype(np.float32)
    return out, res


def kernel(**inputs) -> np.ndarray:
    out, _ = _run(inputs, trace=False)
    return out
